# revision 51
# baseline (speedup 1.0000x reference)
"""Causal self-attention (B=4, T=2048, C=1024, NH=16) on 8 TRN2 NeuronCores.

Sharding (per spec hint): tensor-parallel over heads x data-parallel over batch.
Core i handles batch b = i//2 and head-group g = i%2 (8 heads each).
  - c_attn column-parallel: each core computes q,k,v for its 8 heads.
  - attention: fully local per core (its heads, its batch element).
  - c_proj row-parallel: each core computes a partial (T,C) output from its
    512 features; a 2-core ReduceScatter over pairs [[0,1],[2,3],[4,5],[6,7]]
    sums the partials, each core keeping half the rows. Host concatenates.

Device algorithm (per core), all matmuls bf16 with fp32 PSUM accumulation:
  xT (C,T) staged transposed by host.
  qT = wq^T @ xT, kT = wk^T @ xT   (feature-major, 4 chunks of 128)
  v  = x @ wv                      (token-major) + ones column per head
  per head pair (2fc, 2fc+1), per q-block Q (512 wide):
    s^T[kchunk] = kT_h^T @ qT_h    (K=64 contraction, row-tiled pair -> concurrent)
    p = exp(0.125 * s^T)  (ScalarE, bf16 out); causal-zeroed on GpSimd for
        diagonal chunks; fully-masked chunks skipped entirely; diagonal
        chunks column-trimmed to their live query range on PE/ScalarE/GpSimd.
    o^T[65,512] += v_aug_h^T @ p   (v_aug has a ones column -> row 64 = softmax
        denominators, fused into the same matmul)
    yT_h = o^T[0:64] * (1/o^T[64])  (DVE reciprocal + DMA broadcast)
  partial[T-block] = yT^T @ wp + 0.5*b_proj, streamed straight to HBM in bf16;
  the pairwise sum happens on the host during unsharding (no collectives).

Scheduling: blocks 0-1 are PE-bound, blocks 2-3 exp-bound, so all deferrable
PE work (qkv second halves, v units, c_proj blocks) is injected into blocks
2-3 INSIDE the chunk loop, where the 3-deep exp backlog keeps ScalarE
saturated across each burst.
"""

import sys

if "/opt/trn_rl_repo" not in sys.path:
    sys.path.insert(0, "/opt/trn_rl_repo")

import numpy as np
import ml_dtypes

import concourse.bass as bass
import concourse.bacc as bacc
import concourse.mybir as mybir
import concourse.tile as tile
from concourse.bass import ts, ds
from concourse.bass_utils import run_bass_kernel_spmd

BF16 = ml_dtypes.bfloat16
N_CORES = 8
B, T, C = 4, 2048, 1024
NH, HS = 16, 64
H_LOC = NH // 2        # heads per core
F = H_LOC * HS         # 512 local qkv features
NFC = F // 128         # 4 feature chunks (one head pair each)
NKC = T // 128         # 16 key chunks
NQ = T // 512          # 4 query blocks
NCOL = C // 512        # 2 output column blocks
REPLICA_GROUPS = [[0, 1], [2, 3], [4, 5], [6, 7]]

FP32 = mybir.dt.float32
BF = mybir.dt.bfloat16


def _build_nc():
    # Bacc (not plain Bass): its compile() pipeline runs
    # generate_event_semaphores, which splits sync waits so no instruction
    # carries more than the hardware allows (walrus rejects >1 otherwise).
    nc = bacc.Bacc(None, target_bir_lowering=False, num_devices=N_CORES)

    # All inputs host-restaged so every load DMA reads DRAM sequentially.
    # wq/wk are ko-major so each 128-feature contraction chunk is one
    # contiguous 128KB transfer, letting the first matmuls start as soon as
    # (wq chunk 0, x chunk 0) land instead of after the full weight stream.
    xs = nc.dram_tensor("xs", [128, C // 128, T], BF, kind="ExternalInput")
    wq = nc.dram_tensor("wq", [128, C // 128, F], BF, kind="ExternalInput")
    wk = nc.dram_tensor("wk", [128, C // 128, F], BF, kind="ExternalInput")
    wv = nc.dram_tensor("wv", [128, C // 128, F], BF, kind="ExternalInput")
    bq = nc.dram_tensor("bq", [128, NFC], FP32, kind="ExternalInput")
    bk = nc.dram_tensor("bk", [128, NFC], FP32, kind="ExternalInput")
    bv = nc.dram_tensor("bv", [F], FP32, kind="ExternalInput")
    wp = nc.dram_tensor("wp", [128, NFC, C], BF, kind="ExternalInput")
    bp = nc.dram_tensor("bp", [C], FP32, kind="ExternalInput")
    out = nc.dram_tensor("out", [T, C], BF, kind="ExternalOutput")

    with tile.TileContext(nc) as tc:
        _body(tc, xs, wq, wk, wv, bq, bk, bv, wp, bp, out)
    nc.compile()
    return nc


def _body(tc, xs, wq, wk, wv, bq, bk, bv, wp, bp, out):
    nc = tc.nc
    import contextlib

    ctx = contextlib.ExitStack()
    with ctx:
        wpool = ctx.enter_context(tc.tile_pool(name="weights", bufs=1))
        apool = ctx.enter_context(tc.tile_pool(name="acts", bufs=1))
        ppool = ctx.enter_context(tc.tile_pool(name="ptiles", bufs=3))
        npool = ctx.enter_context(tc.tile_pool(name="norm", bufs=3))
        outp = ctx.enter_context(tc.tile_pool(name="outsb", bufs=4))
        # PSUM budget (8 banks): sAB [128,1024] x3 bufs = 6, oA/oB 1 bank each = 2
        ps_s = ctx.enter_context(tc.tile_pool(name="ps_s", bufs=3, space="PSUM"))
        ps_o = ctx.enter_context(tc.tile_pool(name="ps_o", bufs=1, space="PSUM"))

        # ---- stage inputs into SBUF ----
        # Startup-critical bytes spread over three queues, few big transfers
        # (per-transfer setup is ~2.3us, and sub-4KB partition lines tank DMA
        # packet efficiency): x goes as 4x1MB split sync/scalar (two queues in
        # parallel, 2-chunk granularity for matmul gating), weights stream
        # whole on gpsimd with wq first so the first matmul can start ~11us.
        KO = C // 128  # 8 contraction chunks for the projections

        x_sb = wpool.tile([128, C // 128, T], BF)
        wq_sb = wpool.tile([128, C // 128, F], BF)
        wk_sb = wpool.tile([128, C // 128, F], BF)
        # sync's queue spins up ~2us before scalar/gpsimd: it takes wq (the
        # first-matmul gate) then two late x chunks; scalar takes x chunks
        # 0-1 (the other gate); gpsimd takes the middle x chunks + all
        # remaining weights. x goes as single-chunk transfers so arrivals
        # are staggered ~1-2us apart and wave 1 (which consumes ko in
        # arrival order 0,1,2,3,6,7,4,5) never starves.
        nc.sync.dma_start(out=wq_sb, in_=wq.ap())
        nc.sync.dma_start(out=x_sb[:, 6, :], in_=xs.ap()[:, 6, :])
        nc.sync.dma_start(out=x_sb[:, 7, :], in_=xs.ap()[:, 7, :])
        nc.scalar.dma_start(out=x_sb[:, 0, :], in_=xs.ap()[:, 0, :])
        nc.scalar.dma_start(out=x_sb[:, 1, :], in_=xs.ap()[:, 1, :])
        nc.gpsimd.dma_start(out=x_sb[:, 2, :], in_=xs.ap()[:, 2, :])
        nc.gpsimd.dma_start(out=x_sb[:, 3, :], in_=xs.ap()[:, 3, :])
        nc.gpsimd.dma_start(out=x_sb[:, 4, :], in_=xs.ap()[:, 4, :])
        nc.gpsimd.dma_start(out=x_sb[:, 5, :], in_=xs.ap()[:, 5, :])
        nc.gpsimd.dma_start(out=wk_sb, in_=wk.ap())
        bq_sb = wpool.tile([128, NFC], FP32)
        nc.gpsimd.dma_start(out=bq_sb, in_=bq.ap())
        bk_sb = wpool.tile([128, NFC], FP32)
        nc.gpsimd.dma_start(out=bk_sb, in_=bk.ap())
        wv_sb = wpool.tile([128, C // 128, F], BF)
        nc.gpsimd.dma_start(out=wv_sb, in_=wv.ap())
        # broadcast biases across partitions (for token-major layouts)
        bv_bc = wpool.tile([128, F], FP32)
        nc.gpsimd.dma_start(
            out=bv_bc,
            in_=bass.AP(tensor=bv.ap().tensor, offset=0, ap=[[0, 128], [1, F]]),
        )
        wp_sb = wpool.tile([128, NFC, C], BF)
        nc.gpsimd.dma_start(out=wp_sb, in_=wp.ap())
        bp_bc = wpool.tile([128, C], FP32)
        nc.gpsimd.dma_start(
            out=bp_bc,
            in_=bass.AP(tensor=bp.ap().tensor, offset=0, ap=[[0, 128], [1, C]]),
        )
        # 0/1 staircase (keep where key_row <= query_col), duplicated for the
        # two head halves: blocks 0-1 apply causal zeroing as a DVE multiply
        # with this tile -- their diagonal chunks come early in each fc and
        # would otherwise queue on gpsimd behind the normalization broadcast.
        m01 = wpool.tile([128, 2, 512], BF)
        nc.vector.memset(m01, 1.0)
        nc.gpsimd.affine_select(
            out=m01,
            in_=m01,
            compare_op=mybir.AluOpType.is_ge,
            fill=0.0,
            base=0,
            channel_multiplier=-1,
            pattern=[[0, 2], [1, 512]],
        )


        # ---- persistent activations ----
        qT_sb = apool.tile([128, NFC, T], BF)   # q, feature-major
        kT_sb = apool.tile([128, NFC, T], BF)   # k, feature-major
        # v token-major, 66-stride per head: cols 0:64 = v, col 64 = ones
        v_sb = apool.tile([128, NKC, H_LOC, 66], BF)
        nc.vector.memset(v_sb[:, :, :, 64:65], 1.0)
        yT_sb = apool.tile([128, NFC, T], BF)   # attention out, feature-major

        # ---- qkv projection units (emitted piecemeal: half up front, the
        # rest interleaved into the exp-bound attention phase as PE filler) --
        def qk_half(w_sb, b_sb, dst, fc, tq):
            # finer 512-token unit: smaller PE burst per filler slot, so the
            # ScalarE exp backlog survives the interruption
            ps = ps_s.tile([128, 1024], FP32, tag="sAB")
            for kc in range(KO):
                nc.tensor.matmul(
                    ps[:, 0:512],
                    lhsT=w_sb[:, kc, ts(fc, 128)],
                    rhs=x_sb[:, kc, ts(tq, 512)],
                    start=(kc == 0),
                    stop=(kc == KO - 1),
                )
            nc.vector.tensor_scalar_add(
                out=dst[:, fc, ts(tq, 512)],
                in0=ps[:, 0:512],
                scalar1=b_sb[:, fc : fc + 1],
            )

        def v_unit(tc_i):
            ps = ps_s.tile([128, 1024], FP32, tag="sAB")
            for kc in range(KO):
                nc.tensor.matmul(
                    ps[:, 0:512],
                    lhsT=x_sb[:, kc, ts(tc_i, 128)],
                    rhs=wv_sb[:, kc, :],
                    start=(kc == 0),
                    stop=(kc == KO - 1),
                )
            nc.vector.tensor_add(
                out=v_sb[:, tc_i, :, 0:64],
                in0=ps[:, 0:512].rearrange("p (h f) -> p h f", h=H_LOC),
                in1=bv_bc.rearrange("p (h f) -> p h f", h=H_LOC),
            )

        # prefix: everything attention blocks 0-1 need. Units are emitted in
        # waves of three, interleaved by contraction chunk, so the PE tracks
        # the incoming x stream (three units' worth of matmuls per chunk
        # arrival) instead of serializing unit-by-unit behind the DMA.
        waves = [
            [(wq_sb, bq_sb, qT_sb, 0), (wq_sb, bq_sb, qT_sb, 1),
             (wq_sb, bq_sb, qT_sb, 2)],
            [(wq_sb, bq_sb, qT_sb, 3), (wk_sb, bk_sb, kT_sb, 0),
             (wk_sb, bk_sb, kT_sb, 1)],
            [(wk_sb, bk_sb, kT_sb, 2), (wk_sb, bk_sb, kT_sb, 3)],
        ]
        for wave in waves:
            tiles = [
                ps_s.tile([128, 1024], FP32, tag="sAB", name=f"pref{ui}")
                for ui in range(len(wave))
            ]
            for idx, ko in enumerate((0, 1, 2, 3, 6, 7, 4, 5)):  # arrival order
                for t_, (w_sb, _b, _d, fc) in zip(tiles, wave):
                    for half in range(2):
                        nc.tensor.matmul(
                            t_[:, ts(half, 512)],
                            lhsT=w_sb[:, ko, ts(fc, 128)],
                            rhs=x_sb[:, ko, ds(half * 512, 512)],
                            start=(idx == 0),
                            stop=(idx == KO - 1),
                        )
            for t_, (_w, b_sb, dst, fc) in zip(tiles, wave):
                nc.vector.tensor_scalar_add(
                    out=dst[:, fc, 0:1024], in0=t_, scalar1=b_sb[:, fc : fc + 1]
                )
        for tc_i in range(8):
            v_unit(tc_i)

        # Deferred work rides idle PE slots of the attention phase, balanced
        # against each block's exp budget (exp grows 16/32/48/63us over the
        # four blocks while mandatory QK+AV grows 12/18/27/37us). Deadlines:
        # q tq-slice -> start of its block; k tq-slice -> chunk 4*tq of its
        # block's fc0; v chunk i -> AV of chunk i in its block's fc0; c_proj
        # of block Q -> any time after block Q's last norm.
        filler_by_block = {
            0: [lambda fc=fc: qk_half(wq_sb, bq_sb, qT_sb, fc, 2)
                for fc in range(NFC)],
            1: [lambda i=i: v_unit(i) for i in range(8, 12)]
            + [lambda fc=fc: qk_half(wk_sb, bk_sb, kT_sb, fc, 2) for fc in (0, 1)]
            + [lambda fc=fc: qk_half(wq_sb, bq_sb, qT_sb, fc, 3)
               for fc in range(NFC)],
            2: [lambda: qk_half(wk_sb, bk_sb, kT_sb, 2, 2),
                lambda: qk_half(wk_sb, bk_sb, kT_sb, 3, 2),
                lambda: qk_half(wk_sb, bk_sb, kT_sb, 0, 3),
                lambda: qk_half(wk_sb, bk_sb, kT_sb, 1, 3),
                lambda: qk_half(wk_sb, bk_sb, kT_sb, 2, 3),
                lambda: qk_half(wk_sb, bk_sb, kT_sb, 3, 3),
                lambda: v_unit(12), lambda: v_unit(13)],
            # v14/v15 land in block 3 fc0's first two slots (kc==1, kc==3),
            # safely ahead of fc0's kc14/kc15 AV emissions in the drain loop.
            3: [lambda: v_unit(14), lambda: v_unit(15)],
        }

        def proj_tb(Q, tb):
            # one 128-token block of c_proj partial, in bf16, DMAed straight
            # to the output; the pairwise sum happens on the host during
            # unsharding, so no collective (and no tail reduction) at all.
            trow = Q * 4 + tb
            ps = ps_s.tile([128, 1024], FP32, tag="sAB")
            for ncol in range(NCOL):
                for fc in range(NFC):
                    nc.tensor.matmul(
                        ps[:, ts(ncol, 512)],
                        lhsT=yT_sb[:, fc, ts(trow, 128)],
                        rhs=wp_sb[:, fc, ts(ncol, 512)],
                        start=(fc == 0),
                        stop=(fc == NFC - 1),
                    )
            o_sb = outp.tile([128, 1024], BF, tag="osb")
            nc.vector.tensor_add(out=o_sb, in0=ps, in1=bp_bc)
            nc.sync.dma_start(out=out.ap()[ds(trow * 128, 128), :], in_=o_sb)

        # ---- phase 2+3: attention as ONE flattened software pipeline over
        # (Q, fc, kc): the QK->exp stream never pauses at fc or block
        # boundaries (the old per-fc structure drained LAG AVs + leftover
        # fillers at every boundary, starving the ScalarE ~3us x 16 times).
        # AV matmuls trail the QK/exp stream by LAG steps; each head-pair's
        # normalization fires mid-stream right after its last AV, so the
        # next pair's first AV (LAG steps later) never waits on the psum
        # copies. c_proj token-blocks of completed q-blocks ride the work
        # queue into the exp-bound later blocks.
        proj_by_block = {2: [0, 1], 3: [2]}
        LAG = 5  # ppool holds 6 pAB tiles: the stream + 5 in flight

        steps = [
            (Q, fc, kc)
            for Q in range(NQ)
            for fc in range(NFC)
            for kc in range(4 * Q + 4)
        ]
        pbuf = {}
        o_ps = {}
        filler_q = {Q: list(f) for Q, f in filler_by_block.items()}

        def emit_av(Q, fc, kc):
            # Diagonal chunks (kc >= 4Q, j = kc-4Q) only touch query columns
            # [128j, 512): queries below the chunk's key range are fully
            # masked, so their matmul columns are skipped. The psum
            # accumulation group per column region [128j, 128j+128) ends at
            # diagonal chunk j, so that piece gets stop=True; the rest of the
            # chunk's width continues the group.
            if kc == 0:
                o_ps[(Q, fc)] = (
                    ps_o.tile([65, 512], FP32, tag="oA", name="oA"),
                    ps_o.tile([65, 512], FP32, tag="oB", name="oB"),
                )
            oA, oB = o_ps[(Q, fc)]
            pAB = pbuf.pop((Q, fc, kc))
            j = kc - 4 * Q
            cut = max(0, 128 * j)
            for o_, head in ((oA, 0), (oB, 1)):
                nc.tensor.matmul(
                    o_[:, ds(cut, 512 - cut)],
                    lhsT=v_sb[:, kc, 2 * fc + head, 0:65],
                    rhs=pAB[:, ds(512 * head + cut, 512 - cut)],
                    start=(kc == 0),
                    stop=(j >= 0),
                    skip_group_check=True,
                )

        norm_state = {}

        def norm_pre(Q, fc):
            # normalize part 1: psum->SBUF copies (freeing the oA/oB banks
            # for the next pair's AVs), reciprocal on partition 0, then the
            # gpsimd partition_broadcast. The multiplies are NOT emitted here:
            # they would sit in the DVE queue waiting on the (slow, ~2us)
            # broadcast and block the causal mask-muls behind them.
            oA, oB = o_ps.pop((Q, fc))
            oA_sb = npool.tile([65, 512], BF, tag="oAsb", name="oAsb")
            oB_sb = npool.tile([65, 512], BF, tag="oBsb", name="oBsb")
            nc.vector.tensor_copy(out=oA_sb, in_=oA)
            nc.vector.tensor_copy(out=oB_sb, in_=oB)
            # custom-DVE reciprocal_approx_fast requires fp32 inputs at
            # partition base 0 -- stage both heads' denominator rows there
            rz = npool.tile([1, 1024], FP32, tag="rz", name="rz")
            nc.vector.tensor_copy(out=rz[:, 0:512], in_=oA_sb[64:65, :])
            nc.vector.tensor_copy(out=rz[:, 512:1024], in_=oB_sb[64:65, :])
            r = npool.tile([1, 1024], FP32, tag="r", name="r")
            nc.vector.reciprocal_approx_fast(out=r, in_=rz)
            rb = npool.tile([1, 1024], BF, tag="rb", name="rb")
            nc.vector.tensor_copy(out=rb, in_=r)
            bc = npool.tile([64, 1024], BF, tag="bc", name="bc")
            nc.gpsimd.partition_broadcast(bc, rb)
            norm_state[(Q, fc)] = (oA_sb, oB_sb, bc)

        def norm_post(Q, fc):
            # normalize part 2, on the SAME queue as the broadcast (gpsimd):
            # the multiplies naturally run right after it, and the DVE queue
            # never carries a broadcast-dependent op that would block the
            # causal mask-muls behind it
            if fc == NFC - 1:
                # block Q's yT is now fully written: release any c_proj work
                # that was waiting on it
                proj_ready.add(Q)
                work.extend(pending_proj.pop(Q, []))
            oA_sb, oB_sb, bc = norm_state.pop((Q, fc))
            # head A lives on partitions 0:64 of chunk fc
            nc.gpsimd.tensor_mul(
                out=yT_sb[0:64, fc, ts(Q, 512)], in0=oA_sb[0:64, :],
                in1=bc[:, 0:512],
            )
            # head B must land on partitions 64:128 -> stage + DMA shift
            yB = npool.tile([64, 512], BF, tag="yB", name="yB")
            nc.gpsimd.tensor_mul(out=yB, in0=oB_sb[0:64, :], in1=bc[:, 512:1024])
            nc.sync.dma_start(out=yT_sb[64:128, fc, ts(Q, 512)], in_=yB)

        def retire(s):
            # also fire the norm stages pinned to this step: part 1 right
            # after the pair's last AV, part 2 three steps later
            Qp, fcp, kcp = steps[s]
            emit_av(Qp, fcp, kcp)
            if kcp == 4 * Qp + 3:
                norm_pre(Qp, fcp)
            if s >= 3:
                Qd, fcd, kcd = steps[s - 3]
                if kcd == 4 * Qd + 3 and (Qd, fcd) in norm_state:
                    norm_post(Qd, fcd)

        work = []
        proj_ready = set()
        pending_proj = {}
        for s, (Q, fc, kc) in enumerate(steps):
            if kc == 0:
                # this head-pair's share of deferred work enters the queue;
                # c_proj of block Q-1 waits in pending_proj until that
                # block's last norm_post has written its yT rows (block Q-1's
                # AV/norm pipeline drains LAG steps into block Q)
                fl = filler_q.get(Q, [])
                for _ in range(3):
                    if fl:
                        work.append(fl.pop(0))
                for qp in proj_by_block.get(Q, []):
                    item = lambda qp=qp, fc=fc: proj_tb(qp, fc)
                    if qp in proj_ready:
                        work.append(item)
                    else:
                        pending_proj.setdefault(qp, []).append(item)
            j = kc - 4 * Q
            cut = max(0, 128 * j)  # first live query column
            # heads A and B share one 2-bank psum tile: A in cols 0:512
            # (array rows 0:64), B in 512:1024 (rows 64:128); the row-tiled
            # pair runs concurrently on the PE.
            sAB = ps_s.tile([128, 1024], FP32, tag="sAB")
            nc.tensor.matmul(
                sAB[:, ds(cut, 512 - cut)],
                lhsT=kT_sb[0:64, fc, ts(kc, 128)],
                rhs=qT_sb[0:64, fc, ds(Q * 512 + cut, 512 - cut)],
                start=True,
                stop=True,
                tile_position=(0, 0),
            )
            nc.tensor.matmul(
                sAB[:, ds(512 + cut, 512 - cut)],
                lhsT=kT_sb[64:128, fc, ts(kc, 128)],
                rhs=qT_sb[64:128, fc, ds(Q * 512 + cut, 512 - cut)],
                start=True,
                stop=True,
                tile_position=(64, 0),
            )
            pAB = ppool.tile([128, 1024], BF, tag="pAB", bufs=6)
            nc.scalar.activation(
                out=pAB.rearrange("p (h q) -> p h q", h=2)[:, :, cut:],
                in_=sAB.rearrange("p (h q) -> p h q", h=2)[:, :, cut:],
                func=mybir.ActivationFunctionType.Exp,
                scale=0.125,
            )
            if j >= 0:
                # crosses the causal boundary: zero exp of masked scores
                # (k_global > q_global) for both head halves. Blocks 0-1:
                # DVE mask-multiply (their diagonal chunks come so early each
                # fc that gpsimd would stall them behind the normalization
                # broadcast). Blocks 2-3: gpsimd affine_select (diagonals
                # come >=8 chunks in, long after the broadcast).
                pslice = pAB.rearrange("p (h q) -> p h q", h=2)[:, :, cut:]
                if Q <= 1:
                    nc.vector.tensor_mul(
                        out=pslice, in0=pslice, in1=m01[:, :, 0 : 512 - cut]
                    )
                else:
                    nc.gpsimd.affine_select(
                        out=pslice,
                        in_=pslice,
                        compare_op=mybir.AluOpType.is_ge,
                        fill=0.0,
                        base=0,
                        channel_multiplier=-1,
                        pattern=[[0, 2], [1, 512 - cut]],
                    )
            pbuf[(Q, fc, kc)] = pAB
            if s >= LAG:
                retire(s - LAG)
            if kc % 2 == 1 and work:
                work.pop(0)()
        for s in range(len(steps) - LAG, len(steps)):
            retire(s)
        # tail: pre-accumulate the last block's c_proj over the three
        # already-normalized head pairs for three token blocks -- the PE
        # chews on these while the final pair's normalization chain
        # (reciprocal + broadcast + muls + yB shift) completes; only the fc3
        # contributions + bias/store remain serialized behind it.
        trow0 = (NQ - 1) * 4
        tail_ps = []
        for tb in range(3):
            ps = ps_s.tile([128, 1024], FP32, tag="sAB", name="tailps")
            for ncol in range(NCOL):
                for fc in range(NFC - 1):
                    nc.tensor.matmul(
                        ps[:, ts(ncol, 512)],
                        lhsT=yT_sb[:, fc, ts(trow0 + tb, 128)],
                        rhs=wp_sb[:, fc, ts(ncol, 512)],
                        start=(fc == 0),
                        stop=False,
                        skip_group_check=True,
                    )
            tail_ps.append(ps)
        for key in sorted(norm_state):
            norm_post(*key)
        while work:
            work.pop(0)()
        for tb in range(3):
            ps = tail_ps[tb]
            for ncol in range(NCOL):
                nc.tensor.matmul(
                    ps[:, ts(ncol, 512)],
                    lhsT=yT_sb[:, NFC - 1, ts(trow0 + tb, 128)],
                    rhs=wp_sb[:, NFC - 1, ts(ncol, 512)],
                    start=False,
                    stop=True,
                    skip_group_check=True,
                )
            o_sb = outp.tile([128, 1024], BF, tag="osb", name="osb")
            nc.vector.tensor_add(out=o_sb, in0=ps, in1=bp_bc)
            nc.sync.dma_start(out=out.ap()[ds((trow0 + tb) * 128, 128), :], in_=o_sb)
        proj_tb(NQ - 1, 3)


_NC_CACHE = None


def _get_nc():
    global _NC_CACHE
    if _NC_CACHE is None:
        _NC_CACHE = _build_nc()
    return _NC_CACHE


def kernel(x, w_attn, b_attn, w_proj, b_proj):
    x = np.asarray(x)
    w_attn = np.asarray(w_attn)
    b_attn = np.asarray(b_attn)
    w_proj = np.asarray(w_proj)
    b_proj = np.asarray(b_proj)

    nc = _get_nc()

    def stage_w(w):  # [C, F'] -> [128, C//128, F'] (SBUF layout, contiguous)
        Fp = w.shape[1]
        return np.ascontiguousarray(
            w.reshape(C // 128, 128, Fp).transpose(1, 0, 2)
        ).astype(BF16)

    def stage_w_ko(w):  # [C, F'] -> [C//128, 128, F'] (ko-major DRAM chunks)
        Fp = w.shape[1]
        return np.ascontiguousarray(w.reshape(C // 128, 128, Fp)).astype(BF16)

    in_maps = []
    for i in range(N_CORES):
        b, g = i // 2, i % 2
        xT = x[b].T  # [C, T]
        in_maps.append(
            {
                "xs": np.ascontiguousarray(
                    xT.reshape(C // 128, 128, T).transpose(1, 0, 2)
                ).astype(BF16),
                "wq": stage_w(w_attn[:, g * F : (g + 1) * F]),
                "wk": stage_w(w_attn[:, C + g * F : C + (g + 1) * F]),
                "wv": stage_w(w_attn[:, 2 * C + g * F : 2 * C + (g + 1) * F]),
                "bq": np.ascontiguousarray(
                    b_attn[g * F : (g + 1) * F].reshape(NFC, 128).T
                ).astype(np.float32),
                "bk": np.ascontiguousarray(
                    b_attn[C + g * F : C + (g + 1) * F].reshape(NFC, 128).T
                ).astype(np.float32),
                "bv": np.ascontiguousarray(
                    b_attn[2 * C + g * F : 2 * C + (g + 1) * F]
                ).astype(np.float32),
                "wp": np.ascontiguousarray(
                    w_proj[g * F : (g + 1) * F, :].reshape(NFC, 128, C).transpose(1, 0, 2)
                ).astype(BF16),
                "bp": (b_proj * 0.5).astype(np.float32),
            }
        )

    global _last_in_maps
    _last_in_maps = in_maps  # stashed for external profiling harnesses
    res = run_bass_kernel_spmd(nc, in_maps, core_ids=list(range(N_CORES)))

    # Each core's "out" is its c_proj partial (its 512 features' worth) for
    # the whole sequence; unshard = fp32 pair-sum across the head-groups.
    out = np.empty((B, T, C), dtype=np.float32)
    for b in range(B):
        out[b] = res.results[2 * b]["out"].astype(np.float32)
        out[b] += res.results[2 * b + 1]["out"].astype(np.float32)
    return out



# revision 52
# speedup vs baseline: 1.4190x; 1.4190x over previous
"""Causal self-attention (B=4, T=2048, C=1024, NH=16) on 8 TRN2 NeuronCores.

Sharding (per spec hint): tensor-parallel over heads x data-parallel over batch.
Core i handles batch b = i//2 and head-group g = i%2 (8 heads each).
  - c_attn column-parallel: each core computes q,k,v for its 8 heads.
  - attention: fully local per core (its heads, its batch element).
  - c_proj row-parallel: each core computes a partial (T,C) output from its
    512 features; a 2-core ReduceScatter over pairs [[0,1],[2,3],[4,5],[6,7]]
    sums the partials, each core keeping half the rows. Host concatenates.

Device algorithm (per core), all matmuls bf16 with fp32 PSUM accumulation:
  xT (C,T) staged transposed by host.
  qT = wq^T @ xT, kT = wk^T @ xT   (feature-major, 4 chunks of 128)
  v  = x @ wv                      (token-major) + ones column per head
  per head pair (2fc, 2fc+1), per q-block Q (512 wide):
    s^T[kchunk] = kT_h^T @ qT_h    (K=64 contraction, row-tiled pair -> concurrent)
    p = exp(0.125 * s^T)  (ScalarE, bf16 out); causal-zeroed on GpSimd for
        diagonal chunks; fully-masked chunks skipped entirely; diagonal
        chunks column-trimmed to their live query range on PE/ScalarE/GpSimd.
    o^T[65,512] += v_aug_h^T @ p   (v_aug has a ones column -> row 64 = softmax
        denominators, fused into the same matmul)
    yT_h = o^T[0:64] * (1/o^T[64])  (DVE reciprocal + DMA broadcast)
  partial[T-block] = yT^T @ wp + 0.5*b_proj, streamed straight to HBM in bf16;
  the pairwise sum happens on the host during unsharding (no collectives).

Scheduling: blocks 0-1 are PE-bound, blocks 2-3 exp-bound, so all deferrable
PE work (qkv second halves, v units, c_proj blocks) is injected into blocks
2-3 INSIDE the chunk loop, where the 3-deep exp backlog keeps ScalarE
saturated across each burst.
"""

import sys

if "/opt/trn_rl_repo" not in sys.path:
    sys.path.insert(0, "/opt/trn_rl_repo")

import numpy as np
import ml_dtypes

import concourse.bass as bass
import concourse.bacc as bacc
import concourse.mybir as mybir
import concourse.tile as tile
from concourse.bass import ts, ds
from concourse.bass_utils import run_bass_kernel_spmd

BF16 = ml_dtypes.bfloat16
N_CORES = 8
B, T, C = 4, 2048, 1024
NH, HS = 16, 64
H_LOC = NH // 2        # heads per core
F = H_LOC * HS         # 512 local qkv features
NFC = F // 128         # 4 feature chunks (one head pair each)
NKC = T // 128         # 16 key chunks
NQ = T // 512          # 4 query blocks
NCOL = C // 512        # 2 output column blocks
REPLICA_GROUPS = [[0, 1], [2, 3], [4, 5], [6, 7]]

FP32 = mybir.dt.float32
BF = mybir.dt.bfloat16


def _build_nc():
    # Bacc (not plain Bass): its compile() pipeline runs
    # generate_event_semaphores, which splits sync waits so no instruction
    # carries more than the hardware allows (walrus rejects >1 otherwise).
    nc = bacc.Bacc(None, target_bir_lowering=False, num_devices=N_CORES)

    # All inputs host-restaged so every load DMA reads DRAM sequentially.
    # wq/wk are ko-major so each 128-feature contraction chunk is one
    # contiguous 128KB transfer, letting the first matmuls start as soon as
    # (wq chunk 0, x chunk 0) land instead of after the full weight stream.
    xs = nc.dram_tensor("xs", [128, C // 128, T], BF, kind="ExternalInput")
    wq = nc.dram_tensor("wq", [128, C // 128, F], BF, kind="ExternalInput")
    wk = nc.dram_tensor("wk", [128, C // 128, F], BF, kind="ExternalInput")
    wv = nc.dram_tensor("wv", [128, C // 128, F], BF, kind="ExternalInput")
    bq = nc.dram_tensor("bq", [128, NFC], FP32, kind="ExternalInput")
    bk = nc.dram_tensor("bk", [128, NFC], FP32, kind="ExternalInput")
    bv = nc.dram_tensor("bv", [F], FP32, kind="ExternalInput")
    wp = nc.dram_tensor("wp", [128, NFC, C], BF, kind="ExternalInput")
    bp = nc.dram_tensor("bp", [C], FP32, kind="ExternalInput")
    out = nc.dram_tensor("out", [T, C], BF, kind="ExternalOutput")

    with tile.TileContext(nc) as tc:
        _body(tc, xs, wq, wk, wv, bq, bk, bv, wp, bp, out)
    nc.compile()
    return nc


def _body(tc, xs, wq, wk, wv, bq, bk, bv, wp, bp, out):
    nc = tc.nc
    import contextlib

    ctx = contextlib.ExitStack()
    with ctx:
        wpool = ctx.enter_context(tc.tile_pool(name="weights", bufs=1))
        apool = ctx.enter_context(tc.tile_pool(name="acts", bufs=1))
        ppool = ctx.enter_context(tc.tile_pool(name="ptiles", bufs=3))
        npool = ctx.enter_context(tc.tile_pool(name="norm", bufs=3))
        outp = ctx.enter_context(tc.tile_pool(name="outsb", bufs=4))
        # PSUM budget (8 banks): sAB [128,1024] x3 bufs = 6, oA/oB 1 bank each = 2
        ps_s = ctx.enter_context(tc.tile_pool(name="ps_s", bufs=3, space="PSUM"))
        ps_o = ctx.enter_context(tc.tile_pool(name="ps_o", bufs=1, space="PSUM"))

        # ---- stage inputs into SBUF ----
        # Startup-critical bytes spread over three queues, few big transfers
        # (per-transfer setup is ~2.3us, and sub-4KB partition lines tank DMA
        # packet efficiency): x goes as 4x1MB split sync/scalar (two queues in
        # parallel, 2-chunk granularity for matmul gating), weights stream
        # whole on gpsimd with wq first so the first matmul can start ~11us.
        KO = C // 128  # 8 contraction chunks for the projections

        x_sb = wpool.tile([128, C // 128, T], BF)
        wq_sb = wpool.tile([128, C // 128, F], BF)
        wk_sb = wpool.tile([128, C // 128, F], BF)
        # sync's queue spins up ~2us before scalar/gpsimd: it takes wq (the
        # first-matmul gate) then two late x chunks; scalar takes x chunks
        # 0-1 (the other gate); gpsimd takes the middle x chunks + all
        # remaining weights. x goes as single-chunk transfers so arrivals
        # are staggered ~1-2us apart and wave 1 (which consumes ko in
        # arrival order 0,1,2,3,6,7,4,5) never starves.
        nc.sync.dma_start(out=wq_sb, in_=wq.ap())
        nc.sync.dma_start(out=x_sb[:, 6, :], in_=xs.ap()[:, 6, :])
        nc.sync.dma_start(out=x_sb[:, 7, :], in_=xs.ap()[:, 7, :])
        nc.scalar.dma_start(out=x_sb[:, 0, :], in_=xs.ap()[:, 0, :])
        nc.scalar.dma_start(out=x_sb[:, 1, :], in_=xs.ap()[:, 1, :])
        nc.gpsimd.dma_start(out=x_sb[:, 2, :], in_=xs.ap()[:, 2, :])
        nc.gpsimd.dma_start(out=x_sb[:, 3, :], in_=xs.ap()[:, 3, :])
        nc.gpsimd.dma_start(out=x_sb[:, 4, :], in_=xs.ap()[:, 4, :])
        nc.gpsimd.dma_start(out=x_sb[:, 5, :], in_=xs.ap()[:, 5, :])
        nc.gpsimd.dma_start(out=wk_sb, in_=wk.ap())
        bq_sb = wpool.tile([128, NFC], FP32)
        nc.gpsimd.dma_start(out=bq_sb, in_=bq.ap())
        bk_sb = wpool.tile([128, NFC], FP32)
        nc.gpsimd.dma_start(out=bk_sb, in_=bk.ap())
        wv_sb = wpool.tile([128, C // 128, F], BF)
        nc.gpsimd.dma_start(out=wv_sb, in_=wv.ap())
        # broadcast biases across partitions (for token-major layouts)
        bv_bc = wpool.tile([128, F], FP32)
        nc.gpsimd.dma_start(
            out=bv_bc,
            in_=bass.AP(tensor=bv.ap().tensor, offset=0, ap=[[0, 128], [1, F]]),
        )
        wp_sb = wpool.tile([128, NFC, C], BF)
        nc.gpsimd.dma_start(out=wp_sb, in_=wp.ap())
        bp_bc = wpool.tile([128, C], FP32)
        nc.gpsimd.dma_start(
            out=bp_bc,
            in_=bass.AP(tensor=bp.ap().tensor, offset=0, ap=[[0, 128], [1, C]]),
        )
        # 0/1 staircase (keep where key_row <= query_col), duplicated for the
        # two head halves: blocks 0-1 apply causal zeroing as a DVE multiply
        # with this tile -- their diagonal chunks come early in each fc and
        # would otherwise queue on gpsimd behind the normalization broadcast.
        m01 = wpool.tile([128, 2, 512], BF)
        nc.vector.memset(m01, 1.0)
        nc.gpsimd.affine_select(
            out=m01,
            in_=m01,
            compare_op=mybir.AluOpType.is_ge,
            fill=0.0,
            base=0,
            channel_multiplier=-1,
            pattern=[[0, 2], [1, 512]],
        )


        # ---- persistent activations ----
        qT_sb = apool.tile([128, NFC, T], BF)   # q, feature-major
        kT_sb = apool.tile([128, NFC, T], BF)   # k, feature-major
        # v token-major, 66-stride per head: cols 0:64 = v, col 64 = ones
        v_sb = apool.tile([128, NKC, H_LOC, 66], BF)
        nc.vector.memset(v_sb[:, :, :, 64:65], 1.0)
        yT_sb = apool.tile([128, NFC, T], BF)   # attention out, feature-major

        # ---- qkv projection units (emitted piecemeal: half up front, the
        # rest interleaved into the exp-bound attention phase as PE filler) --
        def qk_half(w_sb, b_sb, dst, fc, tq):
            # finer 512-token unit: smaller PE burst per filler slot, so the
            # ScalarE exp backlog survives the interruption
            ps = ps_s.tile([128, 1024], FP32, tag="sAB")
            for kc in range(KO):
                nc.tensor.matmul(
                    ps[:, 0:512],
                    lhsT=w_sb[:, kc, ts(fc, 128)],
                    rhs=x_sb[:, kc, ts(tq, 512)],
                    start=(kc == 0),
                    stop=(kc == KO - 1),
                )
            nc.vector.tensor_scalar_add(
                out=dst[:, fc, ts(tq, 512)],
                in0=ps[:, 0:512],
                scalar1=b_sb[:, fc : fc + 1],
            )

        def v_unit(tc_i):
            ps = ps_s.tile([128, 1024], FP32, tag="sAB")
            for kc in range(KO):
                nc.tensor.matmul(
                    ps[:, 0:512],
                    lhsT=x_sb[:, kc, ts(tc_i, 128)],
                    rhs=wv_sb[:, kc, :],
                    start=(kc == 0),
                    stop=(kc == KO - 1),
                )
            nc.vector.tensor_add(
                out=v_sb[:, tc_i, :, 0:64],
                in0=ps[:, 0:512].rearrange("p (h f) -> p h f", h=H_LOC),
                in1=bv_bc.rearrange("p (h f) -> p h f", h=H_LOC),
            )

        # prefix: everything attention blocks 0-1 need. Units are emitted in
        # waves of three, interleaved by contraction chunk, so the PE tracks
        # the incoming x stream (three units' worth of matmuls per chunk
        # arrival) instead of serializing unit-by-unit behind the DMA.
        waves = [
            [(wq_sb, bq_sb, qT_sb, 0), (wq_sb, bq_sb, qT_sb, 1),
             (wq_sb, bq_sb, qT_sb, 2)],
            [(wq_sb, bq_sb, qT_sb, 3), (wk_sb, bk_sb, kT_sb, 0),
             (wk_sb, bk_sb, kT_sb, 1)],
            [(wk_sb, bk_sb, kT_sb, 2), (wk_sb, bk_sb, kT_sb, 3)],
        ]
        for wave in waves:
            tiles = [
                ps_s.tile([128, 1024], FP32, tag="sAB", name=f"pref{ui}")
                for ui in range(len(wave))
            ]
            for idx, ko in enumerate((0, 1, 2, 3, 6, 7, 4, 5)):  # arrival order
                for t_, (w_sb, _b, _d, fc) in zip(tiles, wave):
                    for half in range(2):
                        nc.tensor.matmul(
                            t_[:, ts(half, 512)],
                            lhsT=w_sb[:, ko, ts(fc, 128)],
                            rhs=x_sb[:, ko, ds(half * 512, 512)],
                            start=(idx == 0),
                            stop=(idx == KO - 1),
                        )
            for t_, (_w, b_sb, dst, fc) in zip(tiles, wave):
                nc.vector.tensor_scalar_add(
                    out=dst[:, fc, 0:1024], in0=t_, scalar1=b_sb[:, fc : fc + 1]
                )
        for tc_i in range(8):
            v_unit(tc_i)

        # Deferred work rides idle PE slots of the attention phase, balanced
        # against each block's exp budget (exp grows 16/32/48/63us over the
        # four blocks while mandatory QK+AV grows 12/18/27/37us). Deadlines:
        # q tq-slice -> start of its block; k tq-slice -> chunk 4*tq of its
        # block's fc0; v chunk i -> AV of chunk i in its block's fc0; c_proj
        # of block Q -> any time after block Q's last norm.
        filler_by_block = {
            0: [lambda fc=fc: qk_half(wq_sb, bq_sb, qT_sb, fc, 2)
                for fc in range(NFC)],
            1: [lambda i=i: v_unit(i) for i in range(8, 12)]
            + [lambda fc=fc: qk_half(wk_sb, bk_sb, kT_sb, fc, 2) for fc in (0, 1)]
            + [lambda fc=fc: qk_half(wq_sb, bq_sb, qT_sb, fc, 3)
               for fc in range(NFC)],
            2: [lambda: qk_half(wk_sb, bk_sb, kT_sb, 2, 2),
                lambda: qk_half(wk_sb, bk_sb, kT_sb, 3, 2),
                lambda: qk_half(wk_sb, bk_sb, kT_sb, 0, 3),
                lambda: qk_half(wk_sb, bk_sb, kT_sb, 1, 3),
                lambda: qk_half(wk_sb, bk_sb, kT_sb, 2, 3),
                lambda: qk_half(wk_sb, bk_sb, kT_sb, 3, 3),
                lambda: v_unit(12), lambda: v_unit(13)],
            # v14/v15 land in block 3 fc0's first two slots (kc==1, kc==3),
            # safely ahead of fc0's kc14/kc15 AV emissions in the drain loop.
            3: [lambda: v_unit(14), lambda: v_unit(15)],
        }

        def proj_tb(Q, tb):
            # one 128-token block of c_proj partial, in bf16, DMAed straight
            # to the output; the pairwise sum happens on the host during
            # unsharding, so no collective (and no tail reduction) at all.
            trow = Q * 4 + tb
            ps = ps_s.tile([128, 1024], FP32, tag="sAB")
            for ncol in range(NCOL):
                for fc in range(NFC):
                    nc.tensor.matmul(
                        ps[:, ts(ncol, 512)],
                        lhsT=yT_sb[:, fc, ts(trow, 128)],
                        rhs=wp_sb[:, fc, ts(ncol, 512)],
                        start=(fc == 0),
                        stop=(fc == NFC - 1),
                    )
            o_sb = outp.tile([128, 1024], BF, tag="osb")
            nc.vector.tensor_add(out=o_sb, in0=ps, in1=bp_bc)
            nc.sync.dma_start(out=out.ap()[ds(trow * 128, 128), :], in_=o_sb)

        # ---- phase 2+3: attention as ONE flattened software pipeline over
        # (Q, fc, kc): the QK->exp stream never pauses at fc or block
        # boundaries (the old per-fc structure drained LAG AVs + leftover
        # fillers at every boundary, starving the ScalarE ~3us x 16 times).
        # AV matmuls trail the QK/exp stream by LAG steps; each head-pair's
        # normalization fires mid-stream right after its last AV, so the
        # next pair's first AV (LAG steps later) never waits on the psum
        # copies. c_proj token-blocks of completed q-blocks ride the work
        # queue into the exp-bound later blocks.
        proj_by_block = {2: [0, 1], 3: [2]}
        LAG = 5  # ppool holds 6 pAB tiles: the stream + 5 in flight

        steps = [
            (Q, fc, kc)
            for Q in range(NQ)
            for fc in range(NFC)
            for kc in range(4 * Q + 4)
        ]
        pbuf = {}
        o_ps = {}
        filler_q = {Q: list(f) for Q, f in filler_by_block.items()}

        def emit_av(Q, fc, kc):
            # Diagonal chunks (kc >= 4Q, j = kc-4Q) only touch query columns
            # [128j, 512): queries below the chunk's key range are fully
            # masked, so their matmul columns are skipped. The psum
            # accumulation group per column region [128j, 128j+128) ends at
            # diagonal chunk j, so that piece gets stop=True; the rest of the
            # chunk's width continues the group.
            if kc == 0:
                o_ps[(Q, fc)] = (
                    ps_o.tile([65, 512], FP32, tag="oA", name="oA"),
                    ps_o.tile([65, 512], FP32, tag="oB", name="oB"),
                )
            oA, oB = o_ps[(Q, fc)]
            pAB = pbuf.pop((Q, fc, kc))
            j = kc - 4 * Q
            cut = max(0, 128 * j)
            for o_, head in ((oA, 0), (oB, 1)):
                nc.tensor.matmul(
                    o_[:, ds(cut, 512 - cut)],
                    lhsT=v_sb[:, kc, 2 * fc + head, 0:65],
                    rhs=pAB[:, ds(512 * head + cut, 512 - cut)],
                    start=(kc == 0),
                    stop=(j >= 0),
                    skip_group_check=True,
                )

        norm_state = {}

        def norm_pre(Q, fc):
            # normalize part 1: psum->SBUF copies (freeing the oA/oB banks
            # for the next pair's AVs), reciprocal on partition 0, then the
            # gpsimd partition_broadcast. The multiplies are NOT emitted here:
            # they would sit in the DVE queue waiting on the (slow, ~2us)
            # broadcast and block the causal mask-muls behind them.
            oA, oB = o_ps.pop((Q, fc))
            oA_sb = npool.tile([65, 512], BF, tag="oAsb", name="oAsb")
            oB_sb = npool.tile([65, 512], BF, tag="oBsb", name="oBsb")
            nc.vector.tensor_copy(out=oA_sb, in_=oA)
            nc.vector.tensor_copy(out=oB_sb, in_=oB)
            # custom-DVE reciprocal_approx_fast requires fp32 inputs at
            # partition base 0 -- stage both heads' denominator rows there
            rz = npool.tile([1, 1024], FP32, tag="rz", name="rz")
            nc.vector.tensor_copy(out=rz[:, 0:512], in_=oA_sb[64:65, :])
            nc.vector.tensor_copy(out=rz[:, 512:1024], in_=oB_sb[64:65, :])
            r = npool.tile([1, 1024], FP32, tag="r", name="r")
            nc.vector.reciprocal_approx_fast(out=r, in_=rz)
            rb = npool.tile([1, 1024], BF, tag="rb", name="rb")
            nc.vector.tensor_copy(out=rb, in_=r)
            bc = npool.tile([64, 1024], BF, tag="bc", name="bc")
            nc.gpsimd.partition_broadcast(bc, rb)
            norm_state[(Q, fc)] = (oA_sb, oB_sb, bc)

        def norm_post(Q, fc):
            # normalize part 2, emitted several pipeline steps after the
            # broadcast was queued (it has landed by now, so these DVE
            # multiplies never stall the queue). NOTE: these must NOT go on
            # gpsimd -- its tensor_tensor lives in a different ucode library
            # than partition_broadcast, and mixing them forces a ~6us
            # library reload per head pair.
            if fc == NFC - 1:
                # block Q's yT is now fully written: release any c_proj work
                # that was waiting on it
                proj_ready.add(Q)
                work.extend(pending_proj.pop(Q, []))
            oA_sb, oB_sb, bc = norm_state.pop((Q, fc))
            # head A lives on partitions 0:64 of chunk fc
            nc.vector.tensor_mul(
                out=yT_sb[0:64, fc, ts(Q, 512)], in0=oA_sb[0:64, :],
                in1=bc[:, 0:512],
            )
            # head B must land on partitions 64:128 -> stage + DMA shift
            yB = npool.tile([64, 512], BF, tag="yB", name="yB")
            nc.vector.tensor_mul(out=yB, in0=oB_sb[0:64, :], in1=bc[:, 512:1024])
            nc.sync.dma_start(out=yT_sb[64:128, fc, ts(Q, 512)], in_=yB)

        def retire(s):
            # also fire the norm stages pinned to this step: part 1 right
            # after the pair's last AV, part 2 three steps later
            Qp, fcp, kcp = steps[s]
            emit_av(Qp, fcp, kcp)
            if kcp == 4 * Qp + 3:
                norm_pre(Qp, fcp)
            if s >= 3:
                Qd, fcd, kcd = steps[s - 3]
                if kcd == 4 * Qd + 3 and (Qd, fcd) in norm_state:
                    norm_post(Qd, fcd)

        work = []
        proj_ready = set()
        pending_proj = {}
        for s, (Q, fc, kc) in enumerate(steps):
            if kc == 0:
                # this head-pair's share of deferred work enters the queue;
                # c_proj of block Q-1 waits in pending_proj until that
                # block's last norm_post has written its yT rows (block Q-1's
                # AV/norm pipeline drains LAG steps into block Q)
                fl = filler_q.get(Q, [])
                for _ in range(3):
                    if fl:
                        work.append(fl.pop(0))
                for qp in proj_by_block.get(Q, []):
                    item = lambda qp=qp, fc=fc: proj_tb(qp, fc)
                    if qp in proj_ready:
                        work.append(item)
                    else:
                        pending_proj.setdefault(qp, []).append(item)
            j = kc - 4 * Q
            cut = max(0, 128 * j)  # first live query column
            # heads A and B share one 2-bank psum tile: A in cols 0:512
            # (array rows 0:64), B in 512:1024 (rows 64:128); the row-tiled
            # pair runs concurrently on the PE.
            sAB = ps_s.tile([128, 1024], FP32, tag="sAB")
            nc.tensor.matmul(
                sAB[:, ds(cut, 512 - cut)],
                lhsT=kT_sb[0:64, fc, ts(kc, 128)],
                rhs=qT_sb[0:64, fc, ds(Q * 512 + cut, 512 - cut)],
                start=True,
                stop=True,
                tile_position=(0, 0),
            )
            nc.tensor.matmul(
                sAB[:, ds(512 + cut, 512 - cut)],
                lhsT=kT_sb[64:128, fc, ts(kc, 128)],
                rhs=qT_sb[64:128, fc, ds(Q * 512 + cut, 512 - cut)],
                start=True,
                stop=True,
                tile_position=(64, 0),
            )
            pAB = ppool.tile([128, 1024], BF, tag="pAB", bufs=6)
            nc.scalar.activation(
                out=pAB.rearrange("p (h q) -> p h q", h=2)[:, :, cut:],
                in_=sAB.rearrange("p (h q) -> p h q", h=2)[:, :, cut:],
                func=mybir.ActivationFunctionType.Exp,
                scale=0.125,
            )
            if j >= 0:
                # crosses the causal boundary: zero exp of masked scores
                # (k_global > q_global) for both head halves. Blocks 0-1:
                # DVE mask-multiply (their diagonal chunks come so early each
                # fc that gpsimd would stall them behind the normalization
                # broadcast). Blocks 2-3: gpsimd affine_select (diagonals
                # come >=8 chunks in, long after the broadcast).
                pslice = pAB.rearrange("p (h q) -> p h q", h=2)[:, :, cut:]
                if Q <= 1:
                    nc.vector.tensor_mul(
                        out=pslice, in0=pslice, in1=m01[:, :, 0 : 512 - cut]
                    )
                else:
                    nc.gpsimd.affine_select(
                        out=pslice,
                        in_=pslice,
                        compare_op=mybir.AluOpType.is_ge,
                        fill=0.0,
                        base=0,
                        channel_multiplier=-1,
                        pattern=[[0, 2], [1, 512 - cut]],
                    )
            pbuf[(Q, fc, kc)] = pAB
            if s >= LAG:
                retire(s - LAG)
            if kc % 2 == 1 and work:
                work.pop(0)()
        for s in range(len(steps) - LAG, len(steps)):
            retire(s)
        # tail: pre-accumulate the last block's c_proj over the three
        # already-normalized head pairs for three token blocks -- the PE
        # chews on these while the final pair's normalization chain
        # (reciprocal + broadcast + muls + yB shift) completes; only the fc3
        # contributions + bias/store remain serialized behind it.
        trow0 = (NQ - 1) * 4
        tail_ps = []
        for tb in range(3):
            ps = ps_s.tile([128, 1024], FP32, tag="sAB", name="tailps")
            for ncol in range(NCOL):
                for fc in range(NFC - 1):
                    nc.tensor.matmul(
                        ps[:, ts(ncol, 512)],
                        lhsT=yT_sb[:, fc, ts(trow0 + tb, 128)],
                        rhs=wp_sb[:, fc, ts(ncol, 512)],
                        start=(fc == 0),
                        stop=False,
                        skip_group_check=True,
                    )
            tail_ps.append(ps)
        for key in sorted(norm_state):
            norm_post(*key)
        while work:
            work.pop(0)()
        for tb in range(3):
            ps = tail_ps[tb]
            for ncol in range(NCOL):
                nc.tensor.matmul(
                    ps[:, ts(ncol, 512)],
                    lhsT=yT_sb[:, NFC - 1, ts(trow0 + tb, 128)],
                    rhs=wp_sb[:, NFC - 1, ts(ncol, 512)],
                    start=False,
                    stop=True,
                    skip_group_check=True,
                )
            o_sb = outp.tile([128, 1024], BF, tag="osb", name="osb")
            nc.vector.tensor_add(out=o_sb, in0=ps, in1=bp_bc)
            nc.sync.dma_start(out=out.ap()[ds((trow0 + tb) * 128, 128), :], in_=o_sb)
        proj_tb(NQ - 1, 3)


_NC_CACHE = None


def _get_nc():
    global _NC_CACHE
    if _NC_CACHE is None:
        _NC_CACHE = _build_nc()
    return _NC_CACHE


def kernel(x, w_attn, b_attn, w_proj, b_proj):
    x = np.asarray(x)
    w_attn = np.asarray(w_attn)
    b_attn = np.asarray(b_attn)
    w_proj = np.asarray(w_proj)
    b_proj = np.asarray(b_proj)

    nc = _get_nc()

    def stage_w(w):  # [C, F'] -> [128, C//128, F'] (SBUF layout, contiguous)
        Fp = w.shape[1]
        return np.ascontiguousarray(
            w.reshape(C // 128, 128, Fp).transpose(1, 0, 2)
        ).astype(BF16)

    def stage_w_ko(w):  # [C, F'] -> [C//128, 128, F'] (ko-major DRAM chunks)
        Fp = w.shape[1]
        return np.ascontiguousarray(w.reshape(C // 128, 128, Fp)).astype(BF16)

    in_maps = []
    for i in range(N_CORES):
        b, g = i // 2, i % 2
        xT = x[b].T  # [C, T]
        in_maps.append(
            {
                "xs": np.ascontiguousarray(
                    xT.reshape(C // 128, 128, T).transpose(1, 0, 2)
                ).astype(BF16),
                "wq": stage_w(w_attn[:, g * F : (g + 1) * F]),
                "wk": stage_w(w_attn[:, C + g * F : C + (g + 1) * F]),
                "wv": stage_w(w_attn[:, 2 * C + g * F : 2 * C + (g + 1) * F]),
                "bq": np.ascontiguousarray(
                    b_attn[g * F : (g + 1) * F].reshape(NFC, 128).T
                ).astype(np.float32),
                "bk": np.ascontiguousarray(
                    b_attn[C + g * F : C + (g + 1) * F].reshape(NFC, 128).T
                ).astype(np.float32),
                "bv": np.ascontiguousarray(
                    b_attn[2 * C + g * F : 2 * C + (g + 1) * F]
                ).astype(np.float32),
                "wp": np.ascontiguousarray(
                    w_proj[g * F : (g + 1) * F, :].reshape(NFC, 128, C).transpose(1, 0, 2)
                ).astype(BF16),
                "bp": (b_proj * 0.5).astype(np.float32),
            }
        )

    global _last_in_maps
    _last_in_maps = in_maps  # stashed for external profiling harnesses
    res = run_bass_kernel_spmd(nc, in_maps, core_ids=list(range(N_CORES)))

    # Each core's "out" is its c_proj partial (its 512 features' worth) for
    # the whole sequence; unshard = fp32 pair-sum across the head-groups.
    out = np.empty((B, T, C), dtype=np.float32)
    for b in range(B):
        out[b] = res.results[2 * b]["out"].astype(np.float32)
        out[b] += res.results[2 * b + 1]["out"].astype(np.float32)
    return out

